# revision 1
# baseline (speedup 1.0000x reference)
"""Multi-head self-attention with SDPA softcap, sharded over 8 NeuronCores.

Sharding: tensor-parallel over heads. Each core owns 2 of the 16 heads:
  - computes q,k,v projections for its head slice (fp32r matmuls),
  - does attention (softcap tanh + softmax) for its heads over both batches,
  - applies its row-slice of the output projection, producing a partial
    [B*S, D] output. Host sums the 8 partials.

All heavy matmuls run in float32r (fp32 with 11-bit mantissa, full PE rate).
Inputs are pre-rounded to fp32r on the host so device rounding is exact.
"""

import sys

if "/opt/trn_rl_repo" not in sys.path:
    sys.path.insert(0, "/opt/trn_rl_repo")

import numpy as np

import concourse.bass as bass
import concourse.bacc as bacc
import concourse.tile as tile
from concourse import mybir
from concourse.bass_utils import run_bass_kernel_spmd
from concourse.masks import make_identity

F32 = mybir.dt.float32
F32R = mybir.dt.float32r

D = 2048          # model dim
H = 16            # total heads
DK = 128          # head dim
B = 2
S = 2048
T = B * S         # 4096 total tokens
NCORES = 8
HC = 2            # heads per core
DPC = HC * DK     # 256: d' slice per core

KC = D // 128     # 16 contraction chunks over model dim
TCOL = 512        # phase-1 token-column width
NTCOL = T // TCOL             # 16
TQ = 256          # phase-2 query-column width
NTQ = S // TQ                 # 8 per batch
NTK = S // 128    # 16 key blocks per batch


def _round_fp32r(x: np.ndarray) -> np.ndarray:
    """Round fp32 to fp32r (11-bit mantissa), round-to-nearest-even."""
    u = np.ascontiguousarray(x, dtype=np.float32).view(np.uint32)
    low = u & np.uint32(0xFFF)
    kept = u & np.uint32(0xFFFFF000)
    half = np.uint32(0x800)
    roundup = (low > half) | ((low == half) & ((kept & np.uint32(0x1000)) != 0))
    out = kept + np.where(roundup, np.uint32(0x1000), np.uint32(0))
    return out.view(np.float32)

def _build_program(cap: float):
    nc = bacc.Bacc("TRN2", target_bir_lowering=False, debug=False,
                   num_devices=NCORES)

    xT = nc.dram_tensor("xT", [D, T], F32R, kind="ExternalInput").ap()
    ones_d = nc.dram_tensor("ones", [128, 128], F32R, kind="ExternalInput").ap()
    wqT = nc.dram_tensor("wqT", [D, DPC], F32R, kind="ExternalInput").ap()
    wkT = nc.dram_tensor("wkT", [D, DPC], F32R, kind="ExternalInput").ap()
    wvT = nc.dram_tensor("wvT", [D, DPC], F32R, kind="ExternalInput").ap()
    woT = nc.dram_tensor("woT", [DPC, D], F32R, kind="ExternalInput").ap()
    biasT = nc.dram_tensor("biasT", [S, S], F32, kind="ExternalInput").ap()
    out_d = nc.dram_tensor("out_partial", [T, D], F32, kind="ExternalOutput").ap()

    xT_v = xT.rearrange("(kc p) t -> p kc t", p=128)
    biasT_v = biasT.rearrange("(kc p) t -> p kc t", p=128)

    NB = S // 128       # 16 key blocks per batch
    NHF = NTK // 2      # tanh/exp half size (8 key blocks)
    KH = KC // 2

    with tile.TileContext(nc) as tc:
        with (
            tc.tile_pool(name="const", bufs=1) as cpool,
            tc.tile_pool(name="dscr", bufs=1, space="DRAM") as dscr,
            tc.tile_pool(name="wide", bufs=2, space="PSUM") as wide,
            tc.tile_pool(name="spsp", bufs=2, space="PSUM") as spsp,
            tc.tile_pool(name="acc", bufs=2, space="PSUM") as acc,
            tc.tile_pool(name="p2kv", bufs=1) as p2kv,
            tc.tile_pool(name="pqw", bufs=1) as pqw,
        ):
            kT_dram = dscr.tile([HC, 128, T], F32R)       # [h, dk, t]
            v_dram = dscr.tile([T // 128, 128, HC * DK], F32R)  # [tkb, tk%, (h d')]

            ident = cpool.tile([128, 128], F32)
            make_identity(nc, ident[:])
            ones_full = cpool.tile([128, 128], F32R)
            nc.sync.dma_start(out=ones_full[:], in_=ones_d[:])
            wq_sb = pqw.tile([128, KC, DPC], F32R)
            nc.scalar.dma_start(
                out=wq_sb[:], in_=wqT.rearrange("(kc p) n -> p kc n", p=128))

            kv_cur = {}

            def load_kv(b):
                kb = p2kv.tile([128, HC, S], F32R, tag="kb")
                vb = p2kv.tile([128, HC, NB, DK], F32R, tag="vb")
                for h in range(HC):
                    nc.scalar.dma_start(
                        out=kb[:, h, :],
                        in_=kT_dram[h, :, b * S:(b + 1) * S],
                    )
                    nc.scalar.dma_start(
                        out=vb[:, h, :, :],
                        in_=v_dram[b * NB:(b + 1) * NB, :,
                                   h * DK:(h + 1) * DK]
                        .rearrange("a p b -> p a b"),
                    )
                kv_cur[b] = (kb, vb)

            # ---------- Phase 1a: k and v projections (all tokens) ----------
            with (
                tc.tile_pool(name="p1w", bufs=1) as p1w,
                tc.tile_pool(name="p1x", bufs=6) as p1x,
                tc.tile_pool(name="p1tmp", bufs=3) as p1tmp,
                tc.tile_pool(name="p1v", bufs=4) as p1v,
            ):
                wk_sb = p1w.tile([128, KC, DPC], F32R)
                wv_sb = p1w.tile([128, KC, DPC], F32R)
                nc.sync.dma_start(
                    out=wk_sb[:], in_=wkT.rearrange("(kc p) n -> p kc n", p=128))
                nc.scalar.dma_start(
                    out=wv_sb[:], in_=wvT.rearrange("(kc p) n -> p kc n", p=128))

                for tcol in range(NTCOL):
                    t0 = tcol * TCOL
                    xcol_a = p1x.tile([128, KH, TCOL], F32R, tag="xcol")
                    xcol_b = p1x.tile([128, KH, TCOL], F32R, tag="xcol")
                    nc.sync.dma_start(
                        out=xcol_a[:], in_=xT_v[:, 0:KH, t0:t0 + TCOL])
                    nc.sync.dma_start(
                        out=xcol_b[:], in_=xT_v[:, KH:KC, t0:t0 + TCOL])

                    # k: stationary weights, transposed output
                    ps = wide.tile([128, HC, TCOL], F32, tag="wide")
                    for m in range(HC):
                        for kc in range(KC):
                            xc = xcol_a if kc < KH else xcol_b
                            nc.tensor.matmul(
                                ps[:, m, :],
                                wk_sb[:, kc, m * 128:(m + 1) * 128],
                                xc[:, kc % KH, :],
                                start=(kc == 0),
                                stop=(kc == KC - 1),
                            )
                    st = p1tmp.tile([128, HC, TCOL], F32R, tag="st")
                    nc.vector.tensor_copy(
                        st[:].rearrange("p a b -> p (a b)"),
                        ps[:].rearrange("p a b -> p (a b)"),
                    )
                    for m in range(HC):
                        nc.gpsimd.dma_start(
                            out=kT_dram[m, :, t0:t0 + TCOL],
                            in_=st[:, m, :],
                        )
                    # v: stationary x chunks -> natural [t, (h d')] layout
                    for tsub in range(TCOL // 128):
                        if tsub % 2 == 0:
                            vp = acc.tile([128, 512], F32, tag="acc")
                        else:
                            vp2 = spsp.tile(
                                [128, 2, TQ], F32, tag="sps", name="vp2")
                            vp = vp2[:].rearrange("p a b -> p (a b)")
                        for kc in range(KC):
                            xc = xcol_a if kc < KH else xcol_b
                            nc.tensor.matmul(
                                vp[:, 0:DPC],
                                xc[:, kc % KH, tsub * 128:(tsub + 1) * 128],
                                wv_sb[:, kc, :],
                                start=(kc == 0),
                                stop=(kc == KC - 1),
                            )
                        vst = p1v.tile([128, DPC], F32R, tag="vst")
                        nc.vector.tensor_copy(vst[:], vp[:, 0:DPC])
                        nc.gpsimd.dma_start(
                            out=v_dram[tcol * (TCOL // 128) + tsub, :, :],
                            in_=vst[:],
                        )
                    if tcol == NTCOL // 2 - 1:
                        load_kv(0)  # b=0 k/v landed; prefetch during back half

            # -------- Mixed: q projection streamed + attention + out-proj ----
            with (
                tc.tile_pool(name="pqx", bufs=2) as pqx,
                tc.tile_pool(name="pqst", bufs=3) as pqst,
                tc.tile_pool(name="p2bias", bufs=2) as p2bias,
                tc.tile_pool(name="p2s", bufs=2) as p2s,
                tc.tile_pool(name="p2er", bufs=2) as p2er,
                tc.tile_pool(name="p2misc", bufs=2) as p2misc,
                tc.tile_pool(name="p2ot", bufs=4) as p2ot,
                tc.tile_pool(name="p3w", bufs=1) as p3w,
                tc.tile_pool(name="p3out", bufs=2) as p3out,
            ):
                wo_sb = p3w.tile([128, HC, 4, 512], F32R)
                for hc in range(HC):
                    nc.scalar.dma_start(
                        out=wo_sb[:, hc, :, :],
                        in_=woT[hc * 128:(hc + 1) * 128, :].rearrange(
                            "p (nc n) -> p nc n", n=512
                        ),
                    )

                q_done = set()
                q_cols = {}
                TCQ = 256  # q-pass token-column width

                def ensure_q(b, tqc):
                    g = (b * S + tqc * TQ) // TCQ
                    if g in q_done:
                        return
                    q_done.add(g)
                    t0 = g * TCQ
                    xa = pqx.tile([128, KH, TCQ], F32R, tag="qx")
                    xb = pqx.tile([128, KH, TCQ], F32R, tag="qx")
                    nc.sync.dma_start(out=xa[:], in_=xT_v[:, 0:KH, t0:t0 + TCQ])
                    nc.sync.dma_start(out=xb[:], in_=xT_v[:, KH:KC, t0:t0 + TCQ])
                    ps = wide.tile([128, HC, TCQ], F32, tag="wide")
                    for m in range(HC):
                        for kc in range(KC):
                            xc = xa if kc < KH else xb
                            nc.tensor.matmul(
                                ps[:, m, :],
                                wq_sb[:, kc, m * 128:(m + 1) * 128],
                                xc[:, kc % KH, :],
                                start=(kc == 0),
                                stop=(kc == KC - 1),
                            )
                    qc = pqst.tile([128, HC, TCQ], F32R, tag="qst")
                    nc.vector.tensor_copy(
                        qc[:].rearrange("p a b -> p (a b)"),
                        ps[:].rearrange("p a b -> p (a b)"),
                    )
                    q_cols[g] = qc

                units = [(b, tqc, h)
                         for b in range(B)
                         for tqc in range(NTQ)
                         for h in range(HC)]
                state = {}
                ot_map = {}
                bias_cur = {}

                def stage_a(i):
                    b, tqc, h = units[i]
                    tg0 = b * S
                    q0 = tqc * TQ
                    ensure_q(b, tqc)
                    if b not in kv_cur:
                        load_kv(b)
                    kb, vb = kv_cur[b]
                    if (b, tqc) not in bias_cur:
                        bc = p2bias.tile([128, NTK, TQ], F32, tag="bias")
                        nc.gpsimd.dma_start(
                            out=bc[:], in_=biasT_v[:, :, q0:q0 + TQ])
                        bias_cur.clear()
                        bias_cur[(b, tqc)] = bc
                    bc_full = bias_cur[(b, tqc)]
                    g = (tg0 + q0) // TCQ
                    qcol = q_cols[g][:, h, :]
                    s_buf = p2s.tile([128, NTK, TQ], F32, tag="s")
                    for tkg in range(NTK // 2):
                        sps = spsp.tile([128, 2, TQ], F32, tag="sps")
                        for tkk in range(2):
                            tkb = tkg * 2 + tkk
                            nc.tensor.matmul(
                                sps[:, tkk, :],
                                kb[:, h, tkb * 128:(tkb + 1) * 128],
                                qcol,
                                start=True,
                                stop=True,
                            )
                        nc.vector.tensor_add(
                            s_buf[:, tkg * 2:(tkg + 1) * 2, :],
                            sps[:],
                            bc_full[:, tkg * 2:(tkg + 1) * 2, :],
                        )
                    state[i] = (s_buf, kv_cur[b])

                def stage_b(i):
                    b, tqc, h = units[i]
                    s_buf, (kb, vb) = state.pop(i)
                    er_buf = p2er.tile([128, NTK, TQ], F32R, tag="er")
                    avp = acc.tile([128, TQ], F32, tag="acc")
                    zb = acc.tile([128, TQ], F32, tag="acc")
                    for half in range(2):
                        hs = slice(half * NHF, (half + 1) * NHF)
                        s_flat = s_buf[:, hs, :].rearrange("p a b -> p (a b)")
                        nc.scalar.activation(
                            s_flat, s_flat,
                            mybir.ActivationFunctionType.Tanh,
                            scale=1.0 / cap,
                        )
                        nc.scalar.activation(
                            er_buf[:, hs, :].rearrange("p a b -> p (a b)"),
                            s_flat,
                            mybir.ActivationFunctionType.Exp,
                            scale=cap,
                        )
                        for tkb in range(half * NHF, (half + 1) * NHF):
                            nc.tensor.matmul(
                                avp[:],
                                vb[:, h, tkb, :],
                                er_buf[:, tkb, :],
                                start=(tkb == 0),
                                stop=(tkb == NTK - 1),
                            )
                            nc.tensor.matmul(
                                zb[:],
                                ones_full[:],
                                er_buf[:, tkb, :],
                                start=(tkb == 0),
                                stop=(tkb == NTK - 1),
                            )
                    recip = p2misc.tile([128, TQ], F32, tag="recip")
                    nc.vector.reciprocal_approx_fast(out=recip[:], in_=zb[:])
                    ot_st = p2ot.tile([128, TQ], F32R, tag="ot")
                    nc.vector.tensor_mul(ot_st[:], avp[:], recip[:])
                    ot_map[(b, tqc, h)] = ot_st

                def phase3_chunks(b, tqc):
                    o0 = ot_map.pop((b, tqc, 0))
                    o1 = ot_map.pop((b, tqc, 1))
                    for tb4 in range(TQ // 128):
                        tb = tqc * (TQ // 128) + tb4
                        for ng in range(2):
                            ps3 = wide.tile([128, 2, 512], F32, tag="wide")
                            for nc2 in range(2):
                                ncol = ng * 2 + nc2
                                for hc, o in ((0, o0), (1, o1)):
                                    nc.tensor.matmul(
                                        ps3[:, nc2, :],
                                        o[:, tb4 * 128:(tb4 + 1) * 128],
                                        wo_sb[:, hc, ncol, :],
                                        start=(hc == 0),
                                        stop=(hc == HC - 1),
                                    )
                            outt = p3out.tile([128, 1024], F32, tag="outt")
                            nc.vector.tensor_copy(
                                outt[:], ps3[:].rearrange("p a b -> p (a b)")
                            )
                            nc.gpsimd.dma_start(
                                out=out_d[b * S + tb * 128:
                                          b * S + (tb + 1) * 128,
                                          ng * 1024:(ng + 1) * 1024],
                                in_=outt[:],
                            )

                stage_a(0)
                for i in range(len(units)):
                    if i + 1 < len(units):
                        stage_a(i + 1)
                    stage_b(i)
                    b, tqc, h = units[i]
                    if h == 1:
                        phase3_chunks(b, tqc)

    nc.compile()
    return nc


_PROGRAM_CACHE: dict = {}


def _get_program(cap: float):
    if cap not in _PROGRAM_CACHE:
        _PROGRAM_CACHE[cap] = _build_program(cap)
    return _PROGRAM_CACHE[cap]


def _prepare_in_maps(inp, wq, wk, wv, wo, attn_bias, softcap):
    x = np.ascontiguousarray(np.asarray(inp, dtype=np.float32)).reshape(T, D)
    xT = _round_fp32r(np.ascontiguousarray(x.T))
    biasT = np.ascontiguousarray(
        np.asarray(attn_bias, dtype=np.float32).reshape(S, S).T
    )
    wq = np.asarray(wq, dtype=np.float32)
    wk = np.asarray(wk, dtype=np.float32)
    wv = np.asarray(wv, dtype=np.float32)
    wo = np.asarray(wo, dtype=np.float32)
    scale = 1.0 / np.sqrt(np.float32(DK))

    in_maps = []
    for c in range(NCORES):
        rows = slice(c * DPC, (c + 1) * DPC)
        in_maps.append({
            "xT": xT,
            "ones": np.ones((128, 128), dtype=np.float32),
            "wqT": _round_fp32r((wq[rows] * scale).T),
            "wkT": _round_fp32r(wk[rows].T),
            "wvT": _round_fp32r(wv[rows].T),
            "woT": _round_fp32r(wo[:, rows].T),
            "biasT": biasT,
        })
    return in_maps


def run(inputs: dict, trace: bool = False):
    """Run the SPMD kernel. Returns (full_output, BassKernelResults)."""
    cap = float(inputs["softcap"])
    nc = _get_program(cap)
    in_maps = _prepare_in_maps(
        inputs["inp"], inputs["wq"], inputs["wk"], inputs["wv"],
        inputs["wo"], inputs["attn_bias"], inputs["softcap"],
    )
    res = run_bass_kernel_spmd(
        nc, in_maps, list(range(NCORES)), trace=trace,
    )
    acc = np.zeros((T, D), dtype=np.float64)
    for c in range(NCORES):
        acc += res.results[c]["out_partial"]
    out = acc.astype(np.float32).reshape(B, S, D)
    return out, res


def kernel(**inputs) -> np.ndarray:
    out, _ = run(inputs, trace=False)
    return out


if __name__ == "__main__":
    rng = np.random.default_rng(0)
    sc = 1.0 / np.sqrt(D)
    inputs = {
        "inp": rng.standard_normal((B, S, D)).astype(np.float32),
        "wq": (rng.standard_normal((D, D)) * sc).astype(np.float32),
        "wk": (rng.standard_normal((D, D)) * sc).astype(np.float32),
        "wv": (rng.standard_normal((D, D)) * sc).astype(np.float32),
        "wo": (rng.standard_normal((D, D)) * sc).astype(np.float32),
        "attn_bias": rng.standard_normal((1, 1, S, S)).astype(np.float32),
        "softcap": 30,
    }
    out = kernel(**inputs)
    print("out", out.shape, out.dtype, float(np.abs(out).max()))



# revision 2
# speedup vs baseline: 1.0005x; 1.0005x over previous
"""Multi-head self-attention with SDPA softcap, sharded over 8 NeuronCores.

Sharding: tensor-parallel over heads. Each core owns 2 of the 16 heads:
  - phase 1: single pass over x computing q, k, v projections for its head
    slice; q/k/v stay SBUF-resident in bf16 (no DRAM spill, x read once),
  - phase 2: attention (softcap tanh + softmax) per (qcol, batch, head)
    unit, fused with the row-slice output projection, producing a partial
    [T, D] bf16 output. Host sums the 8 partials.

Matmuls run in bf16 (full PE rate); softmax denominators via ones-matmul.
"""

import sys

if "/opt/trn_rl_repo" not in sys.path:
    sys.path.insert(0, "/opt/trn_rl_repo")

import ml_dtypes
import numpy as np

import concourse.bass as bass
import concourse.bacc as bacc
import concourse.tile as tile
from concourse import mybir
from concourse.bass_utils import run_bass_kernel_spmd

F32 = mybir.dt.float32
F32R = mybir.dt.float32r
BF16 = mybir.dt.bfloat16

D = 2048          # model dim
H = 16            # total heads
DK = 128          # head dim
B = 2
S = 2048
T = B * S         # 4096 total tokens
NCORES = 8
HC = 2            # heads per core
DPC = HC * DK     # 256: d' slice per core

KC = D // 128     # 16 contraction chunks over model dim
KH = KC // 2      # 8: half of the contraction chunks
TCOL = 512        # phase-1 token-column width
NTCOL = T // TCOL             # 8
TQ = 512          # phase-2 query-column width
NTQ = S // TQ                 # 4 per batch
NTK = S // 128    # 16 key blocks per batch
NBT = T // 128    # 32 token blocks total
NHF = NTK // 2    # 8: half of the key blocks


def _build_program(cap: float):
    nc = bacc.Bacc("TRN2", target_bir_lowering=False, debug=False,
                   num_devices=NCORES)

    xT = nc.dram_tensor("xT", [D, T], BF16, kind="ExternalInput").ap()
    ones_d = nc.dram_tensor("ones", [128, 128], BF16, kind="ExternalInput").ap()
    wqT = nc.dram_tensor("wqT", [D, DPC], BF16, kind="ExternalInput").ap()
    wkT = nc.dram_tensor("wkT", [D, DPC], BF16, kind="ExternalInput").ap()
    wvT = nc.dram_tensor("wvT", [D, DPC], BF16, kind="ExternalInput").ap()
    woT = nc.dram_tensor("woT", [DPC, D], BF16, kind="ExternalInput").ap()
    biasT = nc.dram_tensor("biasT", [S, S], BF16, kind="ExternalInput").ap()
    out_d = nc.dram_tensor("out_partial", [T, D], BF16, kind="ExternalOutput").ap()

    xT_v = xT.rearrange("(kc p) t -> p kc t", p=128)
    biasT_v = biasT.rearrange("(kc p) t -> p kc t", p=128)

    with tile.TileContext(nc) as tc:
        with (
            tc.tile_pool(name="const", bufs=1) as cpool,
            tc.tile_pool(name="pqkv", bufs=1) as pqkv,
            tc.tile_pool(name="pwo", bufs=1) as pwo,
            tc.tile_pool(name="pbias", bufs=2) as pbias,
        ):
            ones_sb = cpool.tile([128, 128], BF16)
            nc.sync.dma_start(out=ones_sb[:], in_=ones_d[:])

            # q/k stored transposed per head: [dk, tokens]; v natural:
            # [token-block, token%128, (h dk)]
            q_sb = pqkv.tile([128, HC, T], BF16)
            k_sb = pqkv.tile([128, HC, T], BF16)
            v_sb = pqkv.tile([128, NBT, DPC], BF16)
            wo_sb = pwo.tile([128, HC, 4, TQ], BF16)
            for hc in range(HC):
                nc.scalar.dma_start(
                    out=wo_sb[:, hc, :, :],
                    in_=woT[hc * 128:(hc + 1) * 128, :].rearrange(
                        "p (ng n) -> p ng n", n=TQ
                    ),
                )

            bias_tiles = {}

            def load_bias(tqc):
                if tqc in bias_tiles or tqc >= NTQ:
                    return
                bt = pbias.tile([128, NTK, TQ], BF16, tag="bias")
                nc.gpsimd.dma_start(
                    out=bt[:], in_=biasT_v[:, :, tqc * TQ:(tqc + 1) * TQ])
                bias_tiles[tqc] = bt

            # ---------- Phase 1: q, k, v projections (single x pass) -------
            with (
                tc.tile_pool(name="p1w", bufs=1) as p1w,
                tc.tile_pool(name="p1x", bufs=4) as p1x,
                tc.tile_pool(name="p1qk", bufs=3, space="PSUM") as p1qk,
                tc.tile_pool(name="p1v", bufs=2, space="PSUM") as p1v,
            ):
                wq_sb = p1w.tile([128, KC, DPC], BF16)
                wk_sb = p1w.tile([128, KC, DPC], BF16)
                wv_sb = p1w.tile([128, KC, DPC], BF16)
                nc.scalar.dma_start(
                    out=wq_sb[:], in_=wqT.rearrange("(kc p) n -> p kc n", p=128))
                nc.scalar.dma_start(
                    out=wk_sb[:], in_=wkT.rearrange("(kc p) n -> p kc n", p=128))
                nc.scalar.dma_start(
                    out=wv_sb[:], in_=wvT.rearrange("(kc p) n -> p kc n", p=128))

                for tcol in range(NTCOL):
                    t0 = tcol * TCOL
                    xa = p1x.tile([128, KH, TCOL], BF16, tag="x")
                    xb = p1x.tile([128, KH, TCOL], BF16, tag="x")
                    nc.sync.dma_start(out=xa[:], in_=xT_v[:, 0:KH, t0:t0 + TCOL])
                    nc.sync.dma_start(out=xb[:], in_=xT_v[:, KH:KC, t0:t0 + TCOL])

                    # q/k: stationary weights, transposed output [dk, tokens]
                    for m in range(4):
                        wsb = wq_sb if m < 2 else wk_sb
                        msl = m % 2
                        dst = q_sb if m < 2 else k_sb
                        ps = p1qk.tile([128, TCOL], F32, tag="qk")
                        for kc in range(KC):
                            xc = xa if kc < KH else xb
                            nc.tensor.matmul(
                                ps[:],
                                wsb[:, kc, msl * 128:(msl + 1) * 128],
                                xc[:, kc % KH, :],
                                start=(kc == 0),
                                stop=(kc == KC - 1),
                            )
                        nc.scalar.copy(dst[:, msl, t0:t0 + TCOL], ps[:])
                    # v: stationary x chunks -> natural [t, (h d')] layout
                    for tsub in range(TCOL // 128):
                        pv = p1v.tile([128, DPC], F32, tag="v")
                        for kc in range(KC):
                            xc = xa if kc < KH else xb
                            nc.tensor.matmul(
                                pv[:],
                                xc[:, kc % KH, tsub * 128:(tsub + 1) * 128],
                                wv_sb[:, kc, :],
                                start=(kc == 0),
                                stop=(kc == KC - 1),
                            )
                        nc.scalar.copy(
                            v_sb[:, tcol * (TCOL // 128) + tsub, :], pv[:])
                    if tcol == 0:
                        load_bias(0)
                    if tcol == 2:
                        load_bias(1)

            # ---------- Phase 2: attention + output projection -------------
            units = [(tqc, b, h)
                     for tqc in range(NTQ)
                     for b in range(B)
                     for h in range(HC)]
            s_map = {}
            ot_map = {}

            with (
                tc.tile_pool(name="p2s", bufs=2) as p2s,
                tc.tile_pool(name="p2er", bufs=2) as p2er,
                tc.tile_pool(name="p2rec", bufs=2) as p2rec,
                tc.tile_pool(name="p2ot", bufs=4) as p2ot,
                tc.tile_pool(name="p2out", bufs=6) as p2out,
                tc.tile_pool(name="psps", bufs=2, space="PSUM") as psps,
                tc.tile_pool(name="pav", bufs=2, space="PSUM") as pav,
                tc.tile_pool(name="ppo", bufs=2, space="PSUM") as ppo,
            ):
                def stage_a(i):
                    tqc, b, h = units[i]
                    if h == 0 and b == 0:
                        load_bias(tqc + 1)
                    bt = bias_tiles[tqc]
                    q0 = tqc * TQ
                    qcol = q_sb[:, h, b * S + q0:b * S + q0 + TQ]
                    s_buf = p2s.tile([128, NTK, TQ], BF16, tag="s")
                    for g in range(NTK // 2):
                        sps = psps.tile([128, 2, TQ], F32, tag="sps")
                        for j in range(2):
                            tkb = g * 2 + j
                            nc.tensor.matmul(
                                sps[:, j, :],
                                k_sb[:, h, b * S + tkb * 128:
                                     b * S + (tkb + 1) * 128],
                                qcol,
                                start=True,
                                stop=True,
                            )
                        nc.vector.tensor_add(
                            s_buf[:, g * 2:(g + 1) * 2, :],
                            sps[:],
                            bt[:, g * 2:(g + 1) * 2, :],
                        )
                    s_map[i] = s_buf

                def stage_b(i):
                    tqc, b, h = units[i]
                    s_buf = s_map.pop(i)
                    er = p2er.tile([128, NTK, TQ], BF16, tag="er")
                    av = pav.tile([128, TQ], F32, tag="av")
                    zp = ppo.tile([128, TQ], F32, tag="po", name="zp")
                    for half in range(2):
                        hs = slice(half * NHF, (half + 1) * NHF)
                        s_flat = s_buf[:, hs, :].rearrange("p a b -> p (a b)")
                        nc.scalar.activation(
                            s_flat, s_flat,
                            mybir.ActivationFunctionType.Tanh,
                            scale=1.0 / cap,
                        )
                        nc.scalar.activation(
                            er[:, hs, :].rearrange("p a b -> p (a b)"),
                            s_flat,
                            mybir.ActivationFunctionType.Exp,
                            scale=cap,
                        )
                        for tkb in range(half * NHF, (half + 1) * NHF):
                            nc.tensor.matmul(
                                av[:],
                                v_sb[:, b * NTK + tkb,
                                     h * DK:(h + 1) * DK],
                                er[:, tkb, :],
                                start=(tkb == 0),
                                stop=(tkb == NTK - 1),
                            )
                            nc.tensor.matmul(
                                zp[:],
                                ones_sb[:],
                                er[:, tkb, :],
                                start=(tkb == 0),
                                stop=(tkb == NTK - 1),
                            )
                    rec = p2rec.tile([128, TQ], F32, tag="rec")
                    nc.vector.reciprocal_approx_fast(out=rec[:], in_=zp[:])
                    ot = p2ot.tile([128, TQ], BF16, tag="ot")
                    nc.vector.tensor_mul(ot[:], av[:], rec[:])
                    ot_map[(b, tqc, h)] = ot

                def phase3(tqc, b):
                    o0 = ot_map.pop((b, tqc, 0))
                    o1 = ot_map.pop((b, tqc, 1))
                    n = 0
                    for tb4 in range(TQ // 128):
                        trow = b * S + (tqc * (TQ // 128) + tb4) * 128
                        for ng in range(4):
                            po = ppo.tile([128, TQ], F32, tag="po", name="po")
                            for hc, o in ((0, o0), (1, o1)):
                                nc.tensor.matmul(
                                    po[:],
                                    o[:, tb4 * 128:(tb4 + 1) * 128],
                                    wo_sb[:, hc, ng, :],
                                    start=(hc == 0),
                                    stop=(hc == HC - 1),
                                )
                            outt = p2out.tile([128, TQ], BF16, tag="outt")
                            if n % 2 == 0:
                                nc.vector.tensor_copy(outt[:], po[:])
                            else:
                                nc.scalar.copy(outt[:], po[:])
                            n += 1
                            nc.sync.dma_start(
                                out=out_d[trow:trow + 128,
                                          ng * TQ:(ng + 1) * TQ],
                                in_=outt[:],
                            )

                stage_a(0)
                stage_a(1)
                for i in range(len(units)):
                    stage_b(i)
                    tqc, b, h = units[i]
                    if i + 2 < len(units):
                        stage_a(i + 2)
                    if h == 1:
                        phase3(tqc, b)

    nc.compile()
    return nc


_PROGRAM_CACHE: dict = {}


def _get_program(cap: float):
    if cap not in _PROGRAM_CACHE:
        _PROGRAM_CACHE[cap] = _build_program(cap)
    return _PROGRAM_CACHE[cap]


def _bf16(x: np.ndarray) -> np.ndarray:
    return np.ascontiguousarray(x, dtype=np.float32).astype(ml_dtypes.bfloat16)


def _prepare_in_maps(inp, wq, wk, wv, wo, attn_bias, softcap):
    x = np.ascontiguousarray(np.asarray(inp, dtype=np.float32)).reshape(T, D)
    xT = _bf16(np.ascontiguousarray(x.T))
    biasT = _bf16(np.ascontiguousarray(
        np.asarray(attn_bias, dtype=np.float32).reshape(S, S).T
    ))
    wq = np.asarray(wq, dtype=np.float32)
    wk = np.asarray(wk, dtype=np.float32)
    wv = np.asarray(wv, dtype=np.float32)
    wo = np.asarray(wo, dtype=np.float32)
    scale = 1.0 / np.sqrt(np.float32(DK))
    ones = np.ones((128, 128), dtype=np.float32).astype(ml_dtypes.bfloat16)

    in_maps = []
    for c in range(NCORES):
        rows = slice(c * DPC, (c + 1) * DPC)
        in_maps.append({
            "xT": xT,
            "ones": ones,
            "wqT": _bf16((wq[rows] * scale).T),
            "wkT": _bf16(wk[rows].T),
            "wvT": _bf16(wv[rows].T),
            "woT": _bf16(wo[:, rows].T),
            "biasT": biasT,
        })
    return in_maps


def run(inputs: dict, trace: bool = False):
    """Run the SPMD kernel. Returns (full_output, BassKernelResults)."""
    cap = float(inputs["softcap"])
    nc = _get_program(cap)
    in_maps = _prepare_in_maps(
        inputs["inp"], inputs["wq"], inputs["wk"], inputs["wv"],
        inputs["wo"], inputs["attn_bias"], inputs["softcap"],
    )
    res = run_bass_kernel_spmd(
        nc, in_maps, list(range(NCORES)), trace=trace,
    )
    acc = np.zeros((T, D), dtype=np.float32)
    for c in range(NCORES):
        acc += res.results[c]["out_partial"].astype(np.float32)
    out = acc.reshape(B, S, D)
    return out, res


def kernel(**inputs) -> np.ndarray:
    out, _ = run(inputs, trace=False)
    return out


if __name__ == "__main__":
    rng = np.random.default_rng(0)
    sc = 1.0 / np.sqrt(D)
    inputs = {
        "inp": rng.standard_normal((B, S, D)).astype(np.float32),
        "wq": (rng.standard_normal((D, D)) * sc).astype(np.float32),
        "wk": (rng.standard_normal((D, D)) * sc).astype(np.float32),
        "wv": (rng.standard_normal((D, D)) * sc).astype(np.float32),
        "wo": (rng.standard_normal((D, D)) * sc).astype(np.float32),
        "attn_bias": rng.standard_normal((1, 1, S, S)).astype(np.float32),
        "softcap": 30,
    }
    out = kernel(**inputs)
    print("out", out.shape, out.dtype, float(np.abs(out).max()))


# revision 13
# speedup vs baseline: 1.0822x; 1.0817x over previous
"""Multi-head self-attention with SDPA softcap, sharded over 8 NeuronCores.

Sharding: tensor-parallel over heads. Each core owns 2 of the 16 heads:
  - phase 1: single pass over x computing q, k, v projections for its head
    slice; q/k/v stay SBUF-resident in bf16 (no DRAM spill, x read once),
  - phase 2: attention (softcap tanh + softmax) per (qcol, batch, head)
    unit, fused with the row-slice output projection, producing a partial
    [T, D] bf16 output. Host sums the 8 partials.

Matmuls run in bf16 (full PE rate); softmax denominators via ones-matmul.
"""

import sys

if "/opt/trn_rl_repo" not in sys.path:
    sys.path.insert(0, "/opt/trn_rl_repo")

import ml_dtypes
import numpy as np

import concourse.bass as bass
import concourse.bacc as bacc
import concourse.tile as tile
from concourse import mybir
from concourse.bass_utils import run_bass_kernel_spmd

F32 = mybir.dt.float32
F32R = mybir.dt.float32r
BF16 = mybir.dt.bfloat16

D = 2048          # model dim
H = 16            # total heads
DK = 128          # head dim
B = 2
S = 2048
T = B * S         # 4096 total tokens
NCORES = 8
HC = 2            # heads per core
DPC = HC * DK     # 256: d' slice per core

KC = D // 128     # 16 contraction chunks over model dim
KH = KC // 2      # 8: half of the contraction chunks
TCOL = 512        # phase-1 token-column width
NTCOL = T // TCOL             # 8
TQ = 512          # phase-2 query-column width
NTQ = S // TQ                 # 4 per batch
NTK = S // 128    # 16 key blocks per batch
NBT = T // 128    # 32 token blocks total
NHF = NTK // 2    # 8: half of the key blocks


def _build_program(cap: float):
    nc = bacc.Bacc("TRN2", target_bir_lowering=False, debug=False,
                   num_devices=NCORES)

    xT = nc.dram_tensor("xT", [D, T], F32R, kind="ExternalInput").ap()
    ones_d = nc.dram_tensor("ones", [128, 128], BF16, kind="ExternalInput").ap()
    wqT = nc.dram_tensor("wqT", [D, DPC], F32R, kind="ExternalInput").ap()
    wkT = nc.dram_tensor("wkT", [D, DPC], F32R, kind="ExternalInput").ap()
    wvT = nc.dram_tensor("wvT", [D, DPC], F32R, kind="ExternalInput").ap()
    woT = nc.dram_tensor("woT", [DPC, D], F32R, kind="ExternalInput").ap()
    biasT = nc.dram_tensor("biasT", [S, S], BF16, kind="ExternalInput").ap()
    out_d = nc.dram_tensor("out_partial", [T, D], F32, kind="ExternalOutput").ap()

    xT_v = xT.rearrange("(kc p) t -> p kc t", p=128)
    biasT_v = biasT.rearrange("(kc p) t -> p kc t", p=128)

    with tile.TileContext(nc) as tc:
        with (
            tc.tile_pool(name="const", bufs=1) as cpool,
            tc.tile_pool(name="pqkv", bufs=1) as pqkv,
            tc.tile_pool(name="pwo", bufs=1) as pwo,
            tc.tile_pool(name="pbias", bufs=2) as pbias,
        ):
            ones_sb = cpool.tile([128, 128], BF16)
            nc.sync.dma_start(out=ones_sb[:], in_=ones_d[:])

            # q/k stored transposed per head: [dk, tokens]; v natural:
            # [token-block, token%128, (h dk)]
            q_sb = pqkv.tile([128, HC, T], BF16)
            k_sb = pqkv.tile([128, HC, T], BF16)
            v_sb = pqkv.tile([128, NBT, DPC], BF16)
            wo_sb = pwo.tile([128, HC, 4, TQ], F32R)
            for hc in range(HC):
                nc.gpsimd.dma_start(
                    out=wo_sb[:, hc, :, :],
                    in_=woT[hc * 128:(hc + 1) * 128, :].rearrange(
                        "p (ng n) -> p ng n", n=TQ
                    ),
                )

            bias_tiles = {}

            def load_bias(tqc):
                if tqc in bias_tiles or tqc >= NTQ:
                    return
                bt = pbias.tile([128, NTK, TQ], BF16, tag="bias")
                nc.gpsimd.dma_start(
                    out=bt[:], in_=biasT_v[:, :, tqc * TQ:(tqc + 1) * TQ])
                bias_tiles[tqc] = bt

            # ---------- Phase 1: q, k, v projections (single x pass) -------
            with (
                tc.tile_pool(name="p1w", bufs=1) as p1w,
                tc.tile_pool(name="p1x", bufs=3) as p1x,
                tc.tile_pool(name="p1qk", bufs=3, space="PSUM") as p1qk,
                tc.tile_pool(name="p1v", bufs=2, space="PSUM") as p1v,
            ):
                wq_sb = p1w.tile([128, KC, DPC], F32R)
                wk_sb = p1w.tile([128, KC, DPC], F32R)
                wv_sb = p1w.tile([128, KC, DPC], F32R)
                nc.sync.dma_start(
                    out=wq_sb[:], in_=wqT.rearrange("(kc p) n -> p kc n", p=128))
                nc.scalar.dma_start(
                    out=wk_sb[:], in_=wkT.rearrange("(kc p) n -> p kc n", p=128))
                nc.gpsimd.dma_start(
                    out=wv_sb[:], in_=wvT.rearrange("(kc p) n -> p kc n", p=128))

                for tcol in range(NTCOL):
                    t0 = tcol * TCOL
                    xa = p1x.tile([128, KH, TCOL], F32R, tag="x")
                    xb = p1x.tile([128, KH, TCOL], F32R, tag="x")
                    nc.sync.dma_start(out=xa[:], in_=xT_v[:, 0:KH, t0:t0 + TCOL])
                    nc.sync.dma_start(out=xb[:], in_=xT_v[:, KH:KC, t0:t0 + TCOL])

                    # q/k: stationary weights, transposed output [dk, tokens]
                    for m in range(4):
                        wsb = wq_sb if m < 2 else wk_sb
                        msl = m % 2
                        dst = q_sb if m < 2 else k_sb
                        ps = p1qk.tile([128, TCOL], F32, tag="qk")
                        for kc in range(KC):
                            xc = xa if kc < KH else xb
                            nc.tensor.matmul(
                                ps[:],
                                wsb[:, kc, msl * 128:(msl + 1) * 128],
                                xc[:, kc % KH, :],
                                start=(kc == 0),
                                stop=(kc == KC - 1),
                            )
                        nc.scalar.copy(dst[:, msl, t0:t0 + TCOL], ps[:])
                    # v: stationary x chunks -> natural [t, (h d')] layout
                    for tsub in range(TCOL // 128):
                        pv = p1v.tile([128, DPC], F32, tag="v")
                        for kc in range(KC):
                            xc = xa if kc < KH else xb
                            nc.tensor.matmul(
                                pv[:],
                                xc[:, kc % KH, tsub * 128:(tsub + 1) * 128],
                                wv_sb[:, kc, :],
                                start=(kc == 0),
                                stop=(kc == KC - 1),
                            )
                        nc.scalar.copy(
                            v_sb[:, tcol * (TCOL // 128) + tsub, :], pv[:])
                    if tcol == 0:
                        load_bias(0)
                    if tcol == 2:
                        load_bias(1)

            # ---------- Phase 2: attention + output projection -------------
            units = [(tqc, b, h)
                     for tqc in range(NTQ)
                     for b in range(B)
                     for h in range(HC)]
            s_map = {}
            ot_map = {}

            with (
                tc.tile_pool(name="p2s", bufs=2) as p2s,
                tc.tile_pool(name="p2er", bufs=3) as p2er,
                tc.tile_pool(name="p2rec", bufs=2) as p2rec,
                tc.tile_pool(name="p2ot", bufs=4) as p2ot,
                tc.tile_pool(name="p2out", bufs=4) as p2out,
                tc.tile_pool(name="psps", bufs=2, space="PSUM") as psps,
                tc.tile_pool(name="pav", bufs=2, space="PSUM") as pav,
                tc.tile_pool(name="ppo", bufs=2, space="PSUM") as ppo,
            ):
                def stage_a(i):
                    tqc, b, h = units[i]
                    if h == 0 and b == 0:
                        load_bias(tqc + 1)
                    bt = bias_tiles[tqc]
                    q0 = tqc * TQ
                    qcol = q_sb[:, h, b * S + q0:b * S + q0 + TQ]
                    s_buf = p2s.tile([128, NTK, TQ], BF16, tag="s")
                    for g in range(NTK // 2):
                        sps = psps.tile([128, 2, TQ], F32, tag="sps")
                        for j in range(2):
                            tkb = g * 2 + j
                            nc.tensor.matmul(
                                sps[:, j, :],
                                k_sb[:, h, b * S + tkb * 128:
                                     b * S + (tkb + 1) * 128],
                                qcol,
                                start=True,
                                stop=True,
                            )
                        nc.vector.tensor_add(
                            s_buf[:, g * 2:(g + 1) * 2, :],
                            sps[:],
                            bt[:, g * 2:(g + 1) * 2, :],
                        )
                    s_map[i] = s_buf

                fin_state = {}

                def stage_b(i):
                    tqc, b, h = units[i]
                    s_buf = s_map.pop(i)
                    er = p2er.tile([128, NTK, TQ], BF16, tag="er")
                    av = pav.tile([128, TQ], F32, tag="av")
                    zp = ppo.tile([128, TQ], F32, tag="po", name="zp")
                    for half in range(2):
                        hs = slice(half * NHF, (half + 1) * NHF)
                        s_flat = s_buf[:, hs, :].rearrange("p a b -> p (a b)")
                        nc.scalar.activation(
                            s_flat, s_flat,
                            mybir.ActivationFunctionType.Tanh,
                            scale=1.0 / cap,
                        )
                        nc.scalar.activation(
                            er[:, hs, :].rearrange("p a b -> p (a b)"),
                            s_flat,
                            mybir.ActivationFunctionType.Exp,
                            scale=cap,
                        )
                        for tkb in range(half * NHF, (half + 1) * NHF):
                            nc.tensor.matmul(
                                av[:],
                                v_sb[:, b * NTK + tkb,
                                     h * DK:(h + 1) * DK],
                                er[:, tkb, :],
                                start=(tkb == 0),
                                stop=(tkb == NTK - 1),
                            )
                            nc.tensor.matmul(
                                zp[:],
                                ones_sb[:],
                                er[:, tkb, :],
                                start=(tkb == 0),
                                stop=(tkb == NTK - 1),
                            )
                    fin_state[i] = (av, zp)

                def stage_b_fin(i):
                    tqc, b, h = units[i]
                    av, zp = fin_state.pop(i)
                    rec = p2rec.tile([128, TQ], F32, tag="rec")
                    nc.vector.reciprocal_approx_fast(out=rec[:], in_=zp[:])
                    ot = p2ot.tile([128, TQ], F32R, tag="ot")
                    nc.vector.tensor_mul(ot[:], av[:], rec[:])
                    ot_map[(b, tqc, h)] = ot

                def phase3(tqc, b):
                    o0 = ot_map.pop((b, tqc, 0))
                    o1 = ot_map.pop((b, tqc, 1))
                    n = 0
                    for tb4 in range(TQ // 128):
                        trow = b * S + (tqc * (TQ // 128) + tb4) * 128
                        for ng in range(4):
                            po = ppo.tile([128, TQ], F32, tag="po", name="po")
                            for hc, o in ((0, o0), (1, o1)):
                                nc.tensor.matmul(
                                    po[:],
                                    o[:, tb4 * 128:(tb4 + 1) * 128],
                                    wo_sb[:, hc, ng, :],
                                    start=(hc == 0),
                                    stop=(hc == HC - 1),
                                )
                            outt = p2out.tile([128, TQ], F32, tag="outt")
                            if n % 2 == 0:
                                nc.vector.tensor_copy(outt[:], po[:])
                            else:
                                nc.scalar.copy(outt[:], po[:])
                            n += 1
                            nc.sync.dma_start(
                                out=out_d[trow:trow + 128,
                                          ng * TQ:(ng + 1) * TQ],
                                in_=outt[:],
                            )

                stage_a(0)
                stage_a(1)
                for i in range(len(units)):
                    stage_b(i)
                    tqc, b, h = units[i]
                    if i + 2 < len(units):
                        stage_a(i + 2)
                    stage_b_fin(i)
                    if h == 1:
                        phase3(tqc, b)

    nc.compile()
    return nc


_PROGRAM_CACHE: dict = {}


def _get_program(cap: float):
    if cap not in _PROGRAM_CACHE:
        _PROGRAM_CACHE[cap] = _build_program(cap)
    return _PROGRAM_CACHE[cap]


def _bf16(x: np.ndarray) -> np.ndarray:
    return np.ascontiguousarray(x, dtype=np.float32).astype(ml_dtypes.bfloat16)


def _round_fp32r(x: np.ndarray) -> np.ndarray:
    """Round fp32 to fp32r (11-bit mantissa), round-to-nearest-even."""
    u = np.ascontiguousarray(x, dtype=np.float32).view(np.uint32)
    low = u & np.uint32(0xFFF)
    kept = u & np.uint32(0xFFFFF000)
    half = np.uint32(0x800)
    roundup = (low > half) | ((low == half) & ((kept & np.uint32(0x1000)) != 0))
    out = kept + np.where(roundup, np.uint32(0x1000), np.uint32(0))
    return out.view(np.float32)


def _prepare_in_maps(inp, wq, wk, wv, wo, attn_bias, softcap):
    x = np.ascontiguousarray(np.asarray(inp, dtype=np.float32)).reshape(T, D)
    xT = _round_fp32r(np.ascontiguousarray(x.T))
    biasT = _bf16(np.ascontiguousarray(
        np.asarray(attn_bias, dtype=np.float32).reshape(S, S).T
    ))
    wq = np.asarray(wq, dtype=np.float32)
    wk = np.asarray(wk, dtype=np.float32)
    wv = np.asarray(wv, dtype=np.float32)
    wo = np.asarray(wo, dtype=np.float32)
    scale = 1.0 / np.sqrt(np.float32(DK))
    ones = np.ones((128, 128), dtype=np.float32).astype(ml_dtypes.bfloat16)

    in_maps = []
    for c in range(NCORES):
        rows = slice(c * DPC, (c + 1) * DPC)
        in_maps.append({
            "xT": xT,
            "ones": ones,
            "wqT": _round_fp32r((wq[rows] * scale).T),
            "wkT": _round_fp32r(wk[rows].T),
            "wvT": _round_fp32r(wv[rows].T),
            "woT": _round_fp32r(wo[:, rows].T),
            "biasT": biasT,
        })
    return in_maps


def run(inputs: dict, trace: bool = False):
    """Run the SPMD kernel. Returns (full_output, BassKernelResults)."""
    cap = float(inputs["softcap"])
    nc = _get_program(cap)
    in_maps = _prepare_in_maps(
        inputs["inp"], inputs["wq"], inputs["wk"], inputs["wv"],
        inputs["wo"], inputs["attn_bias"], inputs["softcap"],
    )
    res = run_bass_kernel_spmd(
        nc, in_maps, list(range(NCORES)), trace=trace,
    )
    acc = np.zeros((T, D), dtype=np.float32)
    for c in range(NCORES):
        acc += res.results[c]["out_partial"].astype(np.float32)
    out = acc.reshape(B, S, D)
    return out, res


def kernel(**inputs) -> np.ndarray:
    out, _ = run(inputs, trace=False)
    return out


if __name__ == "__main__":
    rng = np.random.default_rng(0)
    sc = 1.0 / np.sqrt(D)
    inputs = {
        "inp": rng.standard_normal((B, S, D)).astype(np.float32),
        "wq": (rng.standard_normal((D, D)) * sc).astype(np.float32),
        "wk": (rng.standard_normal((D, D)) * sc).astype(np.float32),
        "wv": (rng.standard_normal((D, D)) * sc).astype(np.float32),
        "wo": (rng.standard_normal((D, D)) * sc).astype(np.float32),
        "attn_bias": rng.standard_normal((1, 1, S, S)).astype(np.float32),
        "softcap": 30,
    }
    out = kernel(**inputs)
    print("out", out.shape, out.dtype, float(np.abs(out).max()))


# revision 15
# speedup vs baseline: 1.1296x; 1.0437x over previous
"""Multi-head self-attention with SDPA softcap, sharded over 8 NeuronCores.

Sharding: tensor-parallel over heads. Each core owns 2 of the 16 heads:
  - phase 1: single pass over x computing q, k, v projections for its head
    slice; q/k/v stay SBUF-resident in bf16 (no DRAM spill, x read once),
  - phase 2: attention (softcap tanh + softmax) per (qcol, batch, head)
    unit, fused with the row-slice output projection, producing a partial
    [T, D] bf16 output. Host sums the 8 partials.

Matmuls run in bf16 (full PE rate); softmax denominators via ones-matmul.
"""

import sys

if "/opt/trn_rl_repo" not in sys.path:
    sys.path.insert(0, "/opt/trn_rl_repo")

import ml_dtypes
import numpy as np

import concourse.bass as bass
import concourse.bacc as bacc
import concourse.tile as tile
from concourse import mybir
from concourse.bass_utils import run_bass_kernel_spmd

F32 = mybir.dt.float32
F32R = mybir.dt.float32r
BF16 = mybir.dt.bfloat16
F16 = mybir.dt.float16

D = 2048          # model dim
H = 16            # total heads
DK = 128          # head dim
B = 2
S = 2048
T = B * S         # 4096 total tokens
NCORES = 8
HC = 2            # heads per core
DPC = HC * DK     # 256: d' slice per core

KC = D // 128     # 16 contraction chunks over model dim
KH = KC // 2      # 8: half of the contraction chunks
TCOL = 512        # phase-1 token-column width
NTCOL = T // TCOL             # 8
TQ = 512          # phase-2 query-column width
NTQ = S // TQ                 # 4 per batch
NTK = S // 128    # 16 key blocks per batch
NBT = T // 128    # 32 token blocks total
NHF = NTK // 2    # 8: half of the key blocks


def _build_program(cap: float):
    nc = bacc.Bacc("TRN2", target_bir_lowering=False, debug=False,
                   num_devices=NCORES)

    xT = nc.dram_tensor("xT", [D, T], F16, kind="ExternalInput").ap()
    ones_d = nc.dram_tensor("ones", [128, 128], BF16, kind="ExternalInput").ap()
    wqT = nc.dram_tensor("wqT", [D, DPC], F16, kind="ExternalInput").ap()
    wkT = nc.dram_tensor("wkT", [D, DPC], F16, kind="ExternalInput").ap()
    wvT = nc.dram_tensor("wvT", [D, DPC], F16, kind="ExternalInput").ap()
    woT = nc.dram_tensor("woT", [DPC, D], F16, kind="ExternalInput").ap()
    biasT = nc.dram_tensor("biasT", [S, S], F16, kind="ExternalInput").ap()
    out_d = nc.dram_tensor("out_partial", [T, D], F32, kind="ExternalOutput").ap()

    xT_v = xT.rearrange("(kc p) t -> p kc t", p=128)
    biasT_v = biasT.rearrange("(kc p) t -> p kc t", p=128)

    with tile.TileContext(nc) as tc:
        with (
            tc.tile_pool(name="const", bufs=1) as cpool,
            tc.tile_pool(name="pqkv", bufs=1) as pqkv,
            tc.tile_pool(name="pwo", bufs=1) as pwo,
            tc.tile_pool(name="pbias", bufs=2) as pbias,
        ):
            ones_sb = cpool.tile([128, 128], BF16)
            nc.sync.dma_start(out=ones_sb[:], in_=ones_d[:])

            # q/k stored transposed per head: [dk, tokens]; v natural:
            # [token-block, token%128, (h dk)]
            q_sb = pqkv.tile([128, HC, T], F16)
            k_sb = pqkv.tile([128, HC, T], F16)
            v_sb = pqkv.tile([128, NBT, DPC], BF16)
            wo_sb = pwo.tile([128, HC, 4, TQ], F16)
            for hc in range(HC):
                nc.gpsimd.dma_start(
                    out=wo_sb[:, hc, :, :],
                    in_=woT[hc * 128:(hc + 1) * 128, :].rearrange(
                        "p (ng n) -> p ng n", n=TQ
                    ),
                )

            bias_tiles = {}

            def load_bias(tqc):
                if tqc in bias_tiles or tqc >= NTQ:
                    return
                bt = pbias.tile([128, NTK, TQ], F16, tag="bias")
                nc.gpsimd.dma_start(
                    out=bt[:], in_=biasT_v[:, :, tqc * TQ:(tqc + 1) * TQ])
                bias_tiles[tqc] = bt

            # ---------- Phase 1: q, k, v projections (single x pass) -------
            with (
                tc.tile_pool(name="p1w", bufs=1) as p1w,
                tc.tile_pool(name="p1x", bufs=3) as p1x,
                tc.tile_pool(name="p1qk", bufs=3, space="PSUM") as p1qk,
                tc.tile_pool(name="p1v", bufs=2, space="PSUM") as p1v,
            ):
                wq_sb = p1w.tile([128, KC, DPC], F16)
                wk_sb = p1w.tile([128, KC, DPC], F16)
                wv_sb = p1w.tile([128, KC, DPC], F16)
                wqv = wqT.rearrange("(kc p) n -> p kc n", p=128)
                nc.sync.dma_start(out=wq_sb[:, 0:KH, :], in_=wqv[:, 0:KH, :])
                nc.scalar.dma_start(out=wq_sb[:, KH:KC, :], in_=wqv[:, KH:KC, :])
                nc.gpsimd.dma_start(
                    out=wk_sb[:], in_=wkT.rearrange("(kc p) n -> p kc n", p=128))
                nc.gpsimd.dma_start(
                    out=wv_sb[:], in_=wvT.rearrange("(kc p) n -> p kc n", p=128))

                for tcol in range(NTCOL):
                    t0 = tcol * TCOL
                    xa = p1x.tile([128, KH, TCOL], F16, tag="x")
                    xb = p1x.tile([128, KH, TCOL], F16, tag="x")
                    nc.sync.dma_start(out=xa[:], in_=xT_v[:, 0:KH, t0:t0 + TCOL])
                    nc.scalar.dma_start(out=xb[:], in_=xT_v[:, KH:KC, t0:t0 + TCOL])

                    # q/k: stationary weights, transposed output [dk, tokens]
                    for m in range(4):
                        wsb = wq_sb if m < 2 else wk_sb
                        msl = m % 2
                        dst = q_sb if m < 2 else k_sb
                        ps = p1qk.tile([128, TCOL], F32, tag="qk")
                        for kc in range(KC):
                            xc = xa if kc < KH else xb
                            nc.tensor.matmul(
                                ps[:],
                                wsb[:, kc, msl * 128:(msl + 1) * 128],
                                xc[:, kc % KH, :],
                                start=(kc == 0),
                                stop=(kc == KC - 1),
                            )
                        nc.scalar.copy(dst[:, msl, t0:t0 + TCOL], ps[:])
                    # v: stationary x chunks -> natural [t, (h d')] layout
                    for tsub in range(TCOL // 128):
                        pv = p1v.tile([128, DPC], F32, tag="v")
                        for kc in range(KC):
                            xc = xa if kc < KH else xb
                            nc.tensor.matmul(
                                pv[:],
                                xc[:, kc % KH, tsub * 128:(tsub + 1) * 128],
                                wv_sb[:, kc, :],
                                start=(kc == 0),
                                stop=(kc == KC - 1),
                            )
                        nc.scalar.copy(
                            v_sb[:, tcol * (TCOL // 128) + tsub, :], pv[:])
                    if tcol == 0:
                        load_bias(0)
                    if tcol == 2:
                        load_bias(1)

            # ---------- Phase 2: attention + output projection -------------
            units = [(tqc, b, h)
                     for tqc in range(NTQ)
                     for b in range(B)
                     for h in range(HC)]
            s_map = {}
            ot_map = {}

            with (
                tc.tile_pool(name="p2s", bufs=3) as p2s,
                tc.tile_pool(name="p2er", bufs=3) as p2er,
                tc.tile_pool(name="p2rec", bufs=2) as p2rec,
                tc.tile_pool(name="p2ot", bufs=4) as p2ot,
                tc.tile_pool(name="p2out", bufs=4) as p2out,
                tc.tile_pool(name="psps", bufs=2, space="PSUM") as psps,
                tc.tile_pool(name="pav", bufs=2, space="PSUM") as pav,
                tc.tile_pool(name="ppo", bufs=2, space="PSUM") as ppo,
            ):
                def stage_a(i):
                    tqc, b, h = units[i]
                    if h == 0 and b == 0:
                        load_bias(tqc + 1)
                    bt = bias_tiles[tqc]
                    q0 = tqc * TQ
                    qcol = q_sb[:, h, b * S + q0:b * S + q0 + TQ]
                    s_buf = p2s.tile([128, NTK, TQ], F16, tag="s")
                    for g in range(NTK // 2):
                        sps = psps.tile([128, 2, TQ], F32, tag="sps")
                        for j in range(2):
                            tkb = g * 2 + j
                            nc.tensor.matmul(
                                sps[:, j, :],
                                k_sb[:, h, b * S + tkb * 128:
                                     b * S + (tkb + 1) * 128],
                                qcol,
                                start=True,
                                stop=True,
                            )
                        nc.vector.tensor_add(
                            s_buf[:, g * 2:(g + 1) * 2, :],
                            sps[:],
                            bt[:, g * 2:(g + 1) * 2, :],
                        )
                    s_map[i] = s_buf

                fin_state = {}

                def stage_b(i):
                    tqc, b, h = units[i]
                    s_buf = s_map.pop(i)
                    er = p2er.tile([128, NTK, TQ], BF16, tag="er")
                    av = pav.tile([128, TQ], F32, tag="av")
                    zp = ppo.tile([128, TQ], F32, tag="po", name="zp")
                    for half in range(2):
                        hs = slice(half * NHF, (half + 1) * NHF)
                        s_flat = s_buf[:, hs, :].rearrange("p a b -> p (a b)")
                        nc.scalar.activation(
                            s_flat, s_flat,
                            mybir.ActivationFunctionType.Tanh,
                            scale=1.0 / cap,
                        )
                        nc.scalar.activation(
                            er[:, hs, :].rearrange("p a b -> p (a b)"),
                            s_flat,
                            mybir.ActivationFunctionType.Exp,
                            scale=cap,
                        )
                        for tkb in range(half * NHF, (half + 1) * NHF):
                            nc.tensor.matmul(
                                av[:],
                                v_sb[:, b * NTK + tkb,
                                     h * DK:(h + 1) * DK],
                                er[:, tkb, :],
                                start=(tkb == 0),
                                stop=(tkb == NTK - 1),
                            )
                            nc.tensor.matmul(
                                zp[:],
                                ones_sb[:],
                                er[:, tkb, :],
                                start=(tkb == 0),
                                stop=(tkb == NTK - 1),
                            )
                    fin_state[i] = (av, zp)

                def stage_b_fin(i):
                    tqc, b, h = units[i]
                    av, zp = fin_state.pop(i)
                    rec = p2rec.tile([128, TQ], F32, tag="rec")
                    nc.vector.reciprocal_approx_fast(out=rec[:], in_=zp[:])
                    ot = p2ot.tile([128, TQ], F16, tag="ot")
                    nc.vector.tensor_mul(ot[:], av[:], rec[:])
                    ot_map[(b, tqc, h)] = ot

                def phase3(tqc, b):
                    o0 = ot_map.pop((b, tqc, 0))
                    o1 = ot_map.pop((b, tqc, 1))
                    n = 0
                    for tb4 in range(TQ // 128):
                        trow = b * S + (tqc * (TQ // 128) + tb4) * 128
                        for ng in range(4):
                            po = ppo.tile([128, TQ], F32, tag="po", name="po")
                            for hc, o in ((0, o0), (1, o1)):
                                nc.tensor.matmul(
                                    po[:],
                                    o[:, tb4 * 128:(tb4 + 1) * 128],
                                    wo_sb[:, hc, ng, :],
                                    start=(hc == 0),
                                    stop=(hc == HC - 1),
                                )
                            outt = p2out.tile([128, TQ], F32, tag="outt")
                            if n % 2 == 0:
                                nc.vector.tensor_copy(outt[:], po[:])
                            else:
                                nc.scalar.copy(outt[:], po[:])
                            n += 1
                            nc.sync.dma_start(
                                out=out_d[trow:trow + 128,
                                          ng * TQ:(ng + 1) * TQ],
                                in_=outt[:],
                            )

                stage_a(0)
                stage_a(1)
                for i in range(len(units)):
                    stage_b(i)
                    tqc, b, h = units[i]
                    if i + 2 < len(units):
                        stage_a(i + 2)
                    stage_b_fin(i)
                    if h == 1:
                        phase3(tqc, b)

    nc.compile()
    return nc


_PROGRAM_CACHE: dict = {}


def _get_program(cap: float):
    if cap not in _PROGRAM_CACHE:
        _PROGRAM_CACHE[cap] = _build_program(cap)
    return _PROGRAM_CACHE[cap]


def _bf16(x: np.ndarray) -> np.ndarray:
    return np.ascontiguousarray(x, dtype=np.float32).astype(ml_dtypes.bfloat16)


def _round_fp32r(x: np.ndarray) -> np.ndarray:
    """Round fp32 to fp32r (11-bit mantissa), round-to-nearest-even."""
    u = np.ascontiguousarray(x, dtype=np.float32).view(np.uint32)
    low = u & np.uint32(0xFFF)
    kept = u & np.uint32(0xFFFFF000)
    half = np.uint32(0x800)
    roundup = (low > half) | ((low == half) & ((kept & np.uint32(0x1000)) != 0))
    out = kept + np.where(roundup, np.uint32(0x1000), np.uint32(0))
    return out.view(np.float32)


def _prepare_in_maps(inp, wq, wk, wv, wo, attn_bias, softcap):
    x = np.ascontiguousarray(np.asarray(inp, dtype=np.float32)).reshape(T, D)
    xT = np.ascontiguousarray(x.T).astype(np.float16)
    biasT = np.ascontiguousarray(
        np.asarray(attn_bias, dtype=np.float32).reshape(S, S).T
    ).astype(np.float16)
    wq = np.asarray(wq, dtype=np.float32)
    wk = np.asarray(wk, dtype=np.float32)
    wv = np.asarray(wv, dtype=np.float32)
    wo = np.asarray(wo, dtype=np.float32)
    scale = 1.0 / np.sqrt(np.float32(DK))
    ones = np.ones((128, 128), dtype=np.float32).astype(ml_dtypes.bfloat16)

    in_maps = []
    for c in range(NCORES):
        rows = slice(c * DPC, (c + 1) * DPC)
        in_maps.append({
            "xT": xT,
            "ones": ones,
            "wqT": (wq[rows] * scale).T.astype(np.float16),
            "wkT": np.ascontiguousarray(wk[rows].T).astype(np.float16),
            "wvT": np.ascontiguousarray(wv[rows].T).astype(np.float16),
            "woT": np.ascontiguousarray(wo[:, rows].T).astype(np.float16),
            "biasT": biasT,
        })
    return in_maps


def run(inputs: dict, trace: bool = False):
    """Run the SPMD kernel. Returns (full_output, BassKernelResults)."""
    cap = float(inputs["softcap"])
    nc = _get_program(cap)
    in_maps = _prepare_in_maps(
        inputs["inp"], inputs["wq"], inputs["wk"], inputs["wv"],
        inputs["wo"], inputs["attn_bias"], inputs["softcap"],
    )
    res = run_bass_kernel_spmd(
        nc, in_maps, list(range(NCORES)), trace=trace,
    )
    acc = np.zeros((T, D), dtype=np.float32)
    for c in range(NCORES):
        acc += res.results[c]["out_partial"].astype(np.float32)
    out = acc.reshape(B, S, D)
    return out, res


def kernel(**inputs) -> np.ndarray:
    out, _ = run(inputs, trace=False)
    return out


if __name__ == "__main__":
    rng = np.random.default_rng(0)
    sc = 1.0 / np.sqrt(D)
    inputs = {
        "inp": rng.standard_normal((B, S, D)).astype(np.float32),
        "wq": (rng.standard_normal((D, D)) * sc).astype(np.float32),
        "wk": (rng.standard_normal((D, D)) * sc).astype(np.float32),
        "wv": (rng.standard_normal((D, D)) * sc).astype(np.float32),
        "wo": (rng.standard_normal((D, D)) * sc).astype(np.float32),
        "attn_bias": rng.standard_normal((1, 1, S, S)).astype(np.float32),
        "softcap": 30,
    }
    out = kernel(**inputs)
    print("out", out.shape, out.dtype, float(np.abs(out).max()))


# revision 18
# speedup vs baseline: 1.2045x; 1.0664x over previous
"""Multi-head self-attention with SDPA softcap, sharded over 8 NeuronCores.

Sharding: tensor-parallel over heads. Each core owns 2 of the 16 heads.
Single fused pass: the q/k/v projections for batch 1 are interleaved with
attention units of batch 0, so the activation/vector engines (softmax
tanh+exp, bias adds) stay busy under the projection matmuls.

Dtypes: fp16 for x/weights/q/k/bias/s (11-bit mantissa keeps softmax
logits accurate), bf16 for v and exp(scores) (range), fp32 PSUM/output.
"""

import sys

if "/opt/trn_rl_repo" not in sys.path:
    sys.path.insert(0, "/opt/trn_rl_repo")

import numpy as np

import concourse.bass as bass
import concourse.bacc as bacc
import concourse.tile as tile
from concourse import mybir
from concourse.bass_utils import run_bass_kernel_spmd

F32 = mybir.dt.float32
F32R = mybir.dt.float32r
BF16 = mybir.dt.bfloat16
F16 = mybir.dt.float16

D = 2048          # model dim
H = 16            # total heads
DK = 128          # head dim
B = 2
S = 2048
T = B * S         # 4096 total tokens
NCORES = 8
HC = 2            # heads per core
DPC = HC * DK     # 256: d' slice per core

KC = D // 128     # 16 contraction chunks over model dim
KH = KC // 2      # 8: half of the contraction chunks
TCOL = 512        # phase-1 token-column width
NTCOL = T // TCOL             # 8
TQ = 256          # query-column width per attention unit
NTQ = S // TQ                 # 8 per batch
NTK = S // 128    # 16 key blocks per batch
NBT = T // 128    # 32 token blocks total
NHF = NTK // 2    # 8: half of the key blocks


def _build_program(cap: float):
    nc = bacc.Bacc("TRN2", target_bir_lowering=False, debug=False,
                   num_devices=NCORES)

    xT = nc.dram_tensor("xT", [D, T], F16, kind="ExternalInput").ap()
    ones_d = nc.dram_tensor("ones", [128, 128], BF16, kind="ExternalInput").ap()
    wqT = nc.dram_tensor("wqT", [D, DPC], F16, kind="ExternalInput").ap()
    wkT = nc.dram_tensor("wkT", [D, DPC], F16, kind="ExternalInput").ap()
    wvT = nc.dram_tensor("wvT", [D, DPC], F16, kind="ExternalInput").ap()
    woT = nc.dram_tensor("woT", [DPC, D], F16, kind="ExternalInput").ap()
    biasT = nc.dram_tensor("biasT", [S, S], F16, kind="ExternalInput").ap()
    out_d = nc.dram_tensor("out_partial", [T, D], F32, kind="ExternalOutput").ap()

    xT_v = xT.rearrange("(kc p) t -> p kc t", p=128)
    biasT_v = biasT.rearrange("(kc p) t -> p kc t", p=128)

    with tile.TileContext(nc) as tc:
        with (
            tc.tile_pool(name="const", bufs=1) as cpool,
            tc.tile_pool(name="pqkv", bufs=1) as pqkv,
            tc.tile_pool(name="pwo", bufs=1) as pwo,
            tc.tile_pool(name="pbias", bufs=3) as pbias,
            tc.tile_pool(name="p1w", bufs=1) as p1w,
            tc.tile_pool(name="p1x", bufs=4) as p1x,
            tc.tile_pool(name="p2s", bufs=3) as p2s,
            tc.tile_pool(name="p2er", bufs=3) as p2er,
            tc.tile_pool(name="p2rec", bufs=2) as p2rec,
            tc.tile_pool(name="p2ot", bufs=6) as p2ot,
            tc.tile_pool(name="p2out", bufs=4) as p2out,
            tc.tile_pool(name="psps", bufs=2, space="PSUM") as psps,
            tc.tile_pool(name="pacc", bufs=2, space="PSUM") as pacc,
            tc.tile_pool(name="pio", bufs=2, space="PSUM") as pio,
        ):
            ones_sb = cpool.tile([128, 128], BF16)
            nc.sync.dma_start(out=ones_sb[:], in_=ones_d[:])

            # q/k stored transposed per head: [dk, tokens]; v natural:
            # [token-block, token%128, (h dk)]
            q_sb = pqkv.tile([128, HC, T], F16)
            k_sb = pqkv.tile([128, HC, T], F16)
            v_sb = pqkv.tile([128, NBT, DPC], BF16)
            wo_sb = pwo.tile([128, HC, 4, 512], F16)

            wq_sb = p1w.tile([128, KC, DPC], F16)
            wk_sb = p1w.tile([128, KC, DPC], F16)
            wv_sb = p1w.tile([128, KC, DPC], F16)
            wqv = wqT.rearrange("(kc p) n -> p kc n", p=128)
            nc.sync.dma_start(out=wq_sb[:, 0:KH, :], in_=wqv[:, 0:KH, :])
            nc.scalar.dma_start(out=wq_sb[:, KH:KC, :], in_=wqv[:, KH:KC, :])
            nc.gpsimd.dma_start(
                out=wk_sb[:], in_=wkT.rearrange("(kc p) n -> p kc n", p=128))
            nc.gpsimd.dma_start(
                out=wv_sb[:], in_=wvT.rearrange("(kc p) n -> p kc n", p=128))

            bias_tiles = {}

            def load_bias(b, tqc):
                key = (b, tqc)
                if key in bias_tiles or tqc >= NTQ or b >= B:
                    return
                bt = pbias.tile([128, NTK, TQ], F16, tag="bias")
                nc.gpsimd.dma_start(
                    out=bt[:], in_=biasT_v[:, :, tqc * TQ:(tqc + 1) * TQ])
                bias_tiles[key] = bt

            ncopy = 0

            def proj_tcol(tcol):
                """Emit q/k/v projection work for one 512-token column."""
                nonlocal ncopy
                t0 = tcol * TCOL
                xa = p1x.tile([128, KH, TCOL], F16, tag="x")
                xb = p1x.tile([128, KH, TCOL], F16, tag="x")
                nc.sync.dma_start(out=xa[:], in_=xT_v[:, 0:KH, t0:t0 + TCOL])
                nc.scalar.dma_start(out=xb[:], in_=xT_v[:, KH:KC, t0:t0 + TCOL])

                # q/k: stationary weights, transposed output [dk, tokens]
                for m in range(4):
                    wsb = wq_sb if m < 2 else wk_sb
                    msl = m % 2
                    dst = q_sb if m < 2 else k_sb
                    ps = pio.tile([128, TCOL], F32, tag="io", name="pqk")
                    for kc in range(KC):
                        xc = xa if kc < KH else xb
                        nc.tensor.matmul(
                            ps[:],
                            wsb[:, kc, msl * 128:(msl + 1) * 128],
                            xc[:, kc % KH, :],
                            start=(kc == 0),
                            stop=(kc == KC - 1),
                        )
                    if ncopy % 2 == 0:
                        nc.scalar.copy(dst[:, msl, t0:t0 + TCOL], ps[:])
                    else:
                        nc.vector.tensor_copy(dst[:, msl, t0:t0 + TCOL], ps[:])
                    ncopy += 1
                # v: stationary x chunks -> natural [t, (h d')] layout
                for tsub in range(TCOL // 128):
                    pv = pacc.tile([128, DPC], F32, tag="acc", name="pv")
                    for kc in range(KC):
                        xc = xa if kc < KH else xb
                        nc.tensor.matmul(
                            pv[:],
                            xc[:, kc % KH, tsub * 128:(tsub + 1) * 128],
                            wv_sb[:, kc, :],
                            start=(kc == 0),
                            stop=(kc == KC - 1),
                        )
                    if ncopy % 2 == 0:
                        nc.scalar.copy(
                            v_sb[:, tcol * (TCOL // 128) + tsub, :], pv[:])
                    else:
                        nc.vector.tensor_copy(
                            v_sb[:, tcol * (TCOL // 128) + tsub, :], pv[:])
                    ncopy += 1

            # units: batch-major so batch-0 units can interleave with the
            # batch-1 projection columns.
            units = [(b, tqc, h)
                     for b in range(B)
                     for tqc in range(NTQ)
                     for h in range(HC)]
            s_map = {}
            fin_state = {}
            ot_map = {}
            nout = 0

            def stage_a(i):
                b, tqc, h = units[i]
                if h == 0 and i + 4 < len(units):
                    load_bias(units[i + 4][0], units[i + 4][1])
                bt = bias_tiles[(b, tqc)]
                q0 = tqc * TQ
                qcol = q_sb[:, h, b * S + q0:b * S + q0 + TQ]
                s_buf = p2s.tile([128, NTK, TQ], F16, tag="s")
                for g in range(NTK // 4):
                    sps = psps.tile([128, 4, TQ], F32, tag="sps")
                    for j in range(4):
                        tkb = g * 4 + j
                        nc.tensor.matmul(
                            sps[:, j, :],
                            k_sb[:, h, b * S + tkb * 128:
                                 b * S + (tkb + 1) * 128],
                            qcol,
                            start=True,
                            stop=True,
                        )
                    nc.vector.tensor_add(
                        s_buf[:, g * 4:(g + 1) * 4, :],
                        sps[:],
                        bt[:, g * 4:(g + 1) * 4, :],
                    )
                s_map[i] = s_buf

            def stage_b(i):
                b, tqc, h = units[i]
                s_buf = s_map.pop(i)
                er = p2er.tile([128, NTK, TQ], BF16, tag="er")
                av = pacc.tile([128, TQ], F32, tag="acc", name="av")
                zp = pio.tile([128, TQ], F32, tag="io", name="zp")
                s_flat = s_buf[:].rearrange("p a b -> p (a b)")
                nc.scalar.activation(
                    s_flat, s_flat,
                    mybir.ActivationFunctionType.Tanh,
                    scale=1.0 / cap,
                )
                nc.scalar.activation(
                    er[:].rearrange("p a b -> p (a b)"),
                    s_flat,
                    mybir.ActivationFunctionType.Exp,
                    scale=cap,
                )
                for tkb in range(NTK):
                    nc.tensor.matmul(
                        av[:],
                        v_sb[:, b * NTK + tkb, h * DK:(h + 1) * DK],
                        er[:, tkb, :],
                        start=(tkb == 0),
                        stop=(tkb == NTK - 1),
                    )
                    nc.tensor.matmul(
                        zp[:],
                        ones_sb[:],
                        er[:, tkb, :],
                        start=(tkb == 0),
                        stop=(tkb == NTK - 1),
                    )
                fin_state[i] = (av, zp)

            def stage_b_fin(i):
                b, tqc, h = units[i]
                av, zp = fin_state.pop(i)
                rec = p2rec.tile([128, TQ], F32, tag="rec")
                nc.vector.reciprocal_approx_fast(out=rec[:], in_=zp[:])
                ot = p2ot.tile([128, TQ], F16, tag="ot")
                nc.vector.tensor_mul(ot[:], av[:], rec[:])
                ot_map[(b, tqc, h)] = ot

            def phase3(b, tqc):
                nonlocal nout
                o0 = ot_map.pop((b, tqc, 0))
                o1 = ot_map.pop((b, tqc, 1))
                for tb4 in range(TQ // 128):
                    trow = b * S + (tqc * (TQ // 128) + tb4) * 128
                    for ng in range(4):
                        po = pio.tile([128, 512], F32, tag="io", name="po")
                        for hc, o in ((0, o0), (1, o1)):
                            nc.tensor.matmul(
                                po[:],
                                o[:, tb4 * 128:(tb4 + 1) * 128],
                                wo_sb[:, hc, ng, :],
                                start=(hc == 0),
                                stop=(hc == HC - 1),
                            )
                        outt = p2out.tile([128, 512], F32, tag="outt")
                        if nout % 2 == 0:
                            nc.vector.tensor_copy(outt[:], po[:])
                        else:
                            nc.scalar.copy(outt[:], po[:])
                        nout += 1
                        nc.sync.dma_start(
                            out=out_d[trow:trow + 128,
                                      ng * 512:(ng + 1) * 512],
                            in_=outt[:],
                        )

            # ---------------- fused emission schedule --------------------
            steps_done = 0

            def unit_steps(n):
                """Advance the unit pipeline by n pipeline steps."""
                nonlocal steps_done
                for _ in range(n):
                    i = steps_done
                    if i >= len(units):
                        return
                    if i == 0:
                        stage_a(0)
                        stage_a(1)
                    stage_b(i)
                    if i + 2 < len(units):
                        stage_a(i + 2)
                    stage_b_fin(i)
                    b, tqc, h = units[i]
                    if h == 1:
                        phase3(b, tqc)
                    steps_done += 1

            proj_tcol(0)
            load_bias(0, 0)
            proj_tcol(1)
            load_bias(0, 1)
            proj_tcol(2)
            nc.gpsimd.dma_start(
                out=wo_sb[:],
                in_=woT.rearrange("(hc p) (ng n) -> p hc ng n", p=128, n=512),
            )
            proj_tcol(3)
            for tcol in range(4, NTCOL):
                proj_tcol(tcol)
                unit_steps(4)
            unit_steps(len(units) - steps_done)

    nc.compile()
    return nc


_PROGRAM_CACHE: dict = {}


def _get_program(cap: float):
    if cap not in _PROGRAM_CACHE:
        _PROGRAM_CACHE[cap] = _build_program(cap)
    return _PROGRAM_CACHE[cap]


def _prepare_in_maps(inp, wq, wk, wv, wo, attn_bias, softcap):
    x = np.ascontiguousarray(np.asarray(inp, dtype=np.float32)).reshape(T, D)
    xT = np.ascontiguousarray(x.T).astype(np.float16)
    biasT = np.ascontiguousarray(
        np.asarray(attn_bias, dtype=np.float32).reshape(S, S).T
    ).astype(np.float16)
    wq = np.asarray(wq, dtype=np.float32)
    wk = np.asarray(wk, dtype=np.float32)
    wv = np.asarray(wv, dtype=np.float32)
    wo = np.asarray(wo, dtype=np.float32)
    scale = 1.0 / np.sqrt(np.float32(DK))
    import ml_dtypes
    ones = np.ones((128, 128), dtype=np.float32).astype(ml_dtypes.bfloat16)

    in_maps = []
    for c in range(NCORES):
        rows = slice(c * DPC, (c + 1) * DPC)
        in_maps.append({
            "xT": xT,
            "ones": ones,
            "wqT": (wq[rows] * scale).T.astype(np.float16),
            "wkT": np.ascontiguousarray(wk[rows].T).astype(np.float16),
            "wvT": np.ascontiguousarray(wv[rows].T).astype(np.float16),
            "woT": np.ascontiguousarray(wo[:, rows].T).astype(np.float16),
            "biasT": biasT,
        })
    return in_maps


def run(inputs: dict, trace: bool = False):
    """Run the SPMD kernel. Returns (full_output, BassKernelResults)."""
    cap = float(inputs["softcap"])
    nc = _get_program(cap)
    in_maps = _prepare_in_maps(
        inputs["inp"], inputs["wq"], inputs["wk"], inputs["wv"],
        inputs["wo"], inputs["attn_bias"], inputs["softcap"],
    )
    res = run_bass_kernel_spmd(
        nc, in_maps, list(range(NCORES)), trace=trace,
    )
    acc = np.zeros((T, D), dtype=np.float32)
    for c in range(NCORES):
        acc += res.results[c]["out_partial"]
    out = acc.reshape(B, S, D)
    return out, res


def kernel(**inputs) -> np.ndarray:
    out, _ = run(inputs, trace=False)
    return out


if __name__ == "__main__":
    rng = np.random.default_rng(0)
    sc = 1.0 / np.sqrt(D)
    inputs = {
        "inp": rng.standard_normal((B, S, D)).astype(np.float32),
        "wq": (rng.standard_normal((D, D)) * sc).astype(np.float32),
        "wk": (rng.standard_normal((D, D)) * sc).astype(np.float32),
        "wv": (rng.standard_normal((D, D)) * sc).astype(np.float32),
        "wo": (rng.standard_normal((D, D)) * sc).astype(np.float32),
        "attn_bias": rng.standard_normal((1, 1, S, S)).astype(np.float32),
        "softcap": 30,
    }
    out = kernel(**inputs)
    print("out", out.shape, out.dtype, float(np.abs(out).max()))
